# revision 34
# baseline (speedup 1.0000x reference)
"""ConvolutionKAN Trainium2 kernel (8-core SPMD, data-parallel over batch).

Math: the reference computes, per conv patch element x (one of 3x3x32 = 288
taps x channels) a cubic B-spline basis beta_0..7(x) on a uniform grid over
[-1, 1], contracts with (spline_kernel * scale_factor), and adds a
silu(x) @ scale_factor term plus bias.

Key identity used here: on the uniform grid, with t = 2.5 x + 2.5 in [0, 5),

    beta_j(x) = (1/6) sum_{i=0..4} (-1)^i C(4,i) relu(t - (j + i - 3))^3

For shift m = j + i - 3 <= 0 the relu never clips on t >= 0, so those terms
are plain cubics in x; terms with m >= 5 are identically zero on t < 5.
Hence every beta_j is an exact linear combination of the 8 per-element
features  [x, x^2, x^3, relu(t-1)^3, relu(t-2)^3, relu(t-3)^3, relu(t-4)^3]
plus a constant.  Folding that linear map into the weights host-side turns
the whole KAN conv into, per conv tap, a dense matmul with K = 8*32 = 256
(7 feature rows + the silu row, times 32 channels) -- the constant row
collapses into the bias.  Per core: compute the 8 features per input pixel,
transpose to [K, pixels] via the PE, then 9 taps x 2 K-chunks of 128 rows
accumulate into PSUM over windows of 4 output rows (N = 496 columns).
"""

import numpy as np
from math import comb

KH = KW = 3
C = 32
FILTERS = 128
B, H, W = 16, 64, 64
OH = OW = 62
IN_SIZE = KH * KW * C  # 288
NCORES = 8
BLOC = B // NCORES  # 2 images per core

_FEATURE_ROWS = 8  # x, x2, x3, R1, R2, R3, R4, silu
_NTAP = KH * KW  # 9
_NCHUNK = 2  # 256 k-rows per tap -> 2 chunks of 128

_program_cache = {}


def _basis_row_map():
    """beta_j = sum_rc Bmat[j, rc] * feature_rc(x) + Bconst[j].

    Feature classes rc: 0:x 1:x^2 2:x^3 3:R1 4:R2 5:R3 6:R4 (silu handled
    separately).  R_m = relu(2.5 x + 2.5 - m)^3.
    """
    Bmat = np.zeros((8, 7), dtype=np.float64)
    Bconst = np.zeros((8,), dtype=np.float64)
    for j in range(8):
        for i in range(5):
            m = j + i - 3
            if m >= 5:
                continue
            cf = (-1) ** i * comb(4, i) / 6.0
            if m >= 1:
                Bmat[j, 2 + m] += cf
            else:
                d = 2.5 - m
                # (2.5 x + d)^3 expanded in x
                Bmat[j, 2] += cf * 2.5**3
                Bmat[j, 1] += cf * 3 * 2.5**2 * d
                Bmat[j, 0] += cf * 3 * 2.5 * d * d
                Bconst[j] += cf * d**3
    return Bmat, Bconst


def _prep_weights(spline_kernel, scale_factor, bias):
    """Returns (wpk [128, 18, 128] fp32, bias_eff [128, 1] fp32).

    wpk[krow, tap*2 + q, o]: krow = (rc - 4*q)*32 + c for feature class rc
    (0..7, 7 = silu), chunk q = rc // 4, tap = di*3 + dj, c = channel.
    """
    Bmat, Bconst = _basis_row_map()
    sk = spline_kernel.astype(np.float64)  # (288, 8, 128)
    sf = scale_factor.astype(np.float64)  # (288, 128)
    w = sk * sf[:, None, :]  # (288, 8, 128)

    # (288, 7, 128): per input element, weight of each feature class
    wrows = np.einsum("jr,ijo->iro", Bmat, w)
    wfull = np.concatenate([wrows, sf[:, None, :]], axis=1)  # (288, 8, 128)
    # -> [tap, c, rc, o] -> [tap, rc, c, o]
    wfull = wfull.reshape(_NTAP, C, _FEATURE_ROWS, FILTERS).transpose(0, 2, 1, 3)
    # krow-major layout [128 krow, 9*2 tapchunk, 128 o]
    wpk = np.zeros((128, _NTAP * 2, FILTERS), dtype=np.float64)
    for tap in range(_NTAP):
        for rc in range(_FEATURE_ROWS):
            q, rloc = divmod(rc, 4)
            wpk[rloc * 32 : (rloc + 1) * 32, tap * 2 + q, :] = wfull[tap, rc]

    bias_eff = bias.astype(np.float64) + np.einsum("j,ijo->o", Bconst, w)
    return (
        np.ascontiguousarray(wpk, dtype=np.float32),
        np.ascontiguousarray(bias_eff[:, None], dtype=np.float32),
    )


def _features_np(x):
    """Per-element features, fp32, matching the device computation.
    x: (..., ) -> (..., 8)"""
    x = x.astype(np.float32)
    t = np.float32(2.5) * x
    feats = [x, x * x, (x * x) * x]
    for m in range(1, 5):
        v = np.maximum(t + np.float32(2.5 - m), np.float32(0.0))
        feats.append((v * v) * v)
    sig = 1.0 / (1.0 + np.exp(-x.astype(np.float64)))
    feats.append((x.astype(np.float64) * sig).astype(np.float32))
    return np.stack(feats, axis=-1)


def reference_sim(inputs, spline_kernel, scale_factor, bias, grid=None):
    """Host numpy simulation of the kernel math (for validation)."""
    wpk, bias_eff = _prep_weights(spline_kernel, scale_factor, bias)
    xb = inputs.astype(np.float32)
    feats = _features_np(xb)  # (B, H, W, 32, 8)
    out = np.zeros((xb.shape[0], OH, OW, FILTERS), dtype=np.float64)
    for di in range(KH):
        for dj in range(KW):
            tap = di * 3 + dj
            f = feats[:, di : di + OH, dj : dj + OW]  # (B, OH, OW, 32, 8)
            for q in range(2):
                wq = wpk[:, tap * 2 + q, :].astype(np.float64)  # (128, 128)
                # krow = rloc*32 + c, rc = q*4 + rloc
                fq = f[..., :, q * 4 : (q + 1) * 4]  # (..., 32, 4) c, rloc
                fq = np.moveaxis(fq, -1, -2).reshape(*f.shape[:3], 128)
                out += fq.astype(np.float64) @ wq
    return (out + bias_eff[:, 0]).astype(np.float32)


def _build_program():
    import concourse.mybir as mybir
    from concourse import bacc
    from concourse.tile import TileContext
    from concourse.masks import make_identity

    FP = mybir.dt.float32
    AF = mybir.ActivationFunctionType

    FPR = mybir.dt.float32r
    nc = bacc.Bacc()
    x_d = nc.dram_tensor("x", [BLOC, H, W, C], FP, kind="ExternalInput")
    w_d = nc.dram_tensor("wpk", [128, _NTAP * 2, FILTERS], FPR, kind="ExternalInput")
    b_d = nc.dram_tensor("bias_eff", [128, 1], FP, kind="ExternalInput")
    o_d = nc.dram_tensor("out", [128, OH, BLOC, OW], FP, kind="ExternalOutput")

    with TileContext(nc) as tc:
        with (
            tc.tile_pool(name="singles", bufs=1) as singles,
            tc.tile_pool(name="xp", bufs=3) as xp,
            tc.tile_pool(name="bp", bufs=2) as bp,
            tc.tile_pool(name="vp", bufs=2) as vp,
            tc.tile_pool(name="op", bufs=2) as op,
            tc.tile_pool(name="pt", bufs=4, space="PSUM") as pt,
            tc.tile_pool(name="po", bufs=2, space="PSUM") as po,
        ):
            # group-0 x loads go first so the sync DMA queue starts them
            # during boot (everything else below can overlap them)
            # issued on the Activation HWDGE queue so the sem threshold the
            # first DVE op waits on covers only these two transfers
            x4_0 = xp.tile([128, 4, C], FP, name="x4_0", tag="x4")
            for im in range(BLOC):
                src0 = x_d[im, 0:4, :, :].rearrange("r x c -> x r c")
                nc.scalar.dma_start(out=x4_0[im * 64 : (im + 1) * 64, :, :], in_=src0)

            ident = singles.tile([128, 128], FP)
            make_identity(nc, ident)
            identr = singles.tile([128, 128], FPR)
            nc.vector.tensor_copy(identr, ident)
            rbias = singles.tile([128, 4], FP)
            for m in range(1, 5):
                nc.gpsimd.memset(rbias[:, m - 1 : m], float(2.5 - m))
            # pre-warm the ACT function tables (Relu/Silu/Identity) so the
            # ~1.3us ACT_TABLE_LOADs happen during boot, off the critical path
            warm = singles.tile([128, 1], FP)
            nc.scalar.activation(warm, rbias[:, 0:1], AF.Relu, bias=rbias[:, 0:1], scale=1.0)
            nc.scalar.activation(warm, rbias[:, 0:1], AF.Silu)
            nc.scalar.activation(warm, rbias[:, 0:1], AF.Identity, bias=rbias[:, 0:1], scale=1.0)
            wt = singles.tile([128, _NTAP * 2, FILTERS], FPR)
            biasT = singles.tile([128, 1], FP)
            # feature-transpose buffers: [krow 128, row 64, img 2, x 64]
            bt0 = singles.tile([128, H, BLOC, 64], FPR)
            bt1 = singles.tile([128, H, BLOC, 64], FPR)
            bts = [bt0, bt1]

            # Phase A (per group of 4 input rows): compute the 8 features per
            # pixel in [pixel, feature*32+c] layout, PE-transpose into
            # bt0/bt1 ([krow, pixels]).
            def phase_a(g):
                if g == 0:
                    x4 = x4_0
                else:
                    x4 = xp.tile([128, 4, C], FP, name=f"x4_{g}", tag="x4")
                    for im in range(BLOC):
                        src = x_d[im, g * 4 : (g + 1) * 4, :, :].rearrange(
                            "r x c -> x r c"
                        )
                        nc.sync.dma_start(
                            out=x4[im * 64 : (im + 1) * 64, :, :], in_=src
                        )
                b4 = bp.tile([128, 4, 256], FPR, name=f"b4_{g}", tag="b4")
                x2t = vp.tile([128, 4, C], FP, name=f"x2t_{g}", tag="x2t")
                V = vp.tile([128, 4, 128], FP, name=f"V_{g}", tag="V")
                V2 = vp.tile([128, 4, 128], FP, name=f"V2_{g}", tag="V2")

                nc.vector.tensor_copy(b4[:, :, 0:32], x4)  # x
                nc.vector.tensor_mul(x2t, x4, x4)
                nc.vector.tensor_copy(b4[:, :, 32:64], x2t)  # x^2
                nc.vector.tensor_mul(b4[:, :, 64:96], x2t, x4)  # x^3
                for m in range(1, 5):
                    nc.scalar.activation(
                        V[:, :, (m - 1) * 32 : m * 32],
                        x4,
                        AF.Relu,
                        bias=rbias[:, m - 1 : m],
                        scale=2.5,
                    )
                nc.vector.tensor_mul(V2, V, V)
                nc.vector.tensor_mul(b4[:, :, 96:224], V2, V)  # R_m^3
                nc.scalar.activation(b4[:, :, 224:256], x4, AF.Silu)

                for r in range(4):
                    row = g * 4 + r
                    for q in range(2):
                        ptile = pt.tile([128, 128], FPR, name=f"pt_{g}_{r}_{q}", tag="pt")
                        nc.tensor.transpose(
                            ptile, b4[:, r, q * 128 : (q + 1) * 128], identr
                        )
                        # psum cols are img*64 + x, same as bt layout
                        dst = bts[q][:, row].rearrange("p i x -> p (i x)")
                        nc.vector.tensor_copy(dst, ptile)

            # Phase B (per group of 4 output rows, N = 4*124 = 496 columns).
            # float32r matmuls: 1 PE cycle/row at N >= 256 (vs 4 for fp32).
            # The last group is shifted to y0 = 58 (recomputing rows 58-59)
            # and stores only its last 2 rows, keeping windows in bounds.
            def phase_b(og):
                y0 = og * 4 if og < 15 else 58
                r0 = 0 if og < 15 else 2
                ps = po.tile([128, 4, 124], FP, name=f"ps_{og}", tag="ps")
                idx = 0
                for di in range(KH):
                    for dj in range(KW):
                        for q in range(2):
                            rhs = bts[q][:, y0 + di : y0 + di + 4, :, dj : dj + 62]
                            nc.tensor.matmul(
                                ps,
                                wt[:, (di * 3 + dj) * 2 + q, :],
                                rhs,
                                start=(idx == 0),
                                stop=(idx == 17),
                            )
                            idx += 1
                ot = op.tile([128, 4, 124], FP, name=f"ot_{og}", tag="ot")
                nc.scalar.activation(
                    ot[:, r0:, :],
                    ps[:, r0:, :],
                    AF.Identity,
                    bias=biasT[:, 0:1],
                    scale=1.0,
                )
                nc.sync.dma_start(out=o_d[:, y0 + r0 : y0 + 4, :, :], in_=ot[:, r0:, :])

            # Interleave: phase_b(og) right after phase_a(og + 1) so the PE
            # has a continuous stream of work (stays HAM-warm).  The weight /
            # bias loads go on the gpsimd (SWDGE) queue after the first x
            # loads so they don't delay group 0 on the sync HWDGE queues.
            for g in range(H // 4):
                phase_a(g)
                if g == 0:
                    nc.gpsimd.dma_start(out=wt, in_=w_d[:, :, :])
                    nc.gpsimd.dma_start(out=biasT, in_=b_d[:, :])
                if g >= 1:
                    phase_b(g - 1)
            phase_b(15)
    nc.compile()
    return nc


def _get_program():
    if "nc" not in _program_cache:
        _program_cache["nc"] = _build_program()
    return _program_cache["nc"]


def run_cores(inputs, spline_kernel, scale_factor, bias, trace=False):
    """Run the SPMD kernel on 8 cores; returns (out, BassKernelResults)."""
    from concourse.bass_utils import run_bass_kernel_spmd

    wpk, bias_eff = _prep_weights(spline_kernel, scale_factor, bias)
    x = np.ascontiguousarray(inputs, dtype=np.float32)
    in_maps = [
        {
            "x": x[i * BLOC : (i + 1) * BLOC],
            "wpk": wpk,
            "bias_eff": bias_eff,
        }
        for i in range(NCORES)
    ]
    nc = _get_program()
    res = run_bass_kernel_spmd(nc, in_maps, list(range(NCORES)), trace=trace)
    out = np.empty((B, OH, OW, FILTERS), dtype=np.float32)
    for i in range(NCORES):
        oc = res.results[i]["out"]  # [128, OH, BLOC, OW]
        out[i * BLOC : (i + 1) * BLOC] = np.transpose(oc, (2, 1, 3, 0))
    return out, res


def kernel(inputs, spline_kernel, scale_factor, bias, grid=None, **_):
    out, _res = run_cores(inputs, spline_kernel, scale_factor, bias, trace=False)
    return out
